# revision 6
# baseline (speedup 1.0000x reference)
"""Trainium2 Bass kernel for CustomSoftmaxExperts (topk_masking).

Math: reference computes softmax over the 64-expert axis, finds the 5th
largest softmax value per row, and keeps values >= max(kth, 0.2).
Since softmax rows sum to 1, at most 4 values can be >= 0.2, so any value
>= 0.2 is automatically within the top-5: the mask reduces EXACTLY to
``softmax >= 0.2`` (verified bit-identical against the jax reference).

Kernel per row (64 contiguous f32 in DRAM):
    e = exp(x)            # no max-subtract needed: |x| <= ~5.5, exp <= ~250
    s = sum(e); r = 1/s
    out  = (e*r >= 0.2) ? e*r : 0     # one fused custom-DVE pass

Sharding: 32*8192 = 262144 rows, data-parallel over 8 cores ->
32768 rows/core (8.39 MB in + 8.39 MB out per core; memory-bound,
per-core HBM roofline ~358 GB/s -> ~47 us).

Layout per core: flat [32768*64] viewed as [128 partitions x 16384],
tiled along free dim.  Engines: ACT exp; DVE segmented reduce_sum
[128,K,64]->[128,K], reciprocal, and ONE fused custom-DVE op
(soft = e*r; out = soft >= 0.2 ? soft : 0) via a runtime-registered
DveOp (SOFTMAX_THR_MASK_ANT) — halves DVE element passes vs separate
mul + scalar_tensor_tensor.
"""

import numpy as np

import concourse.bacc as bacc
import concourse.mybir as mybir
from concourse import bass_utils, dve_ops
from concourse.dve_spec import (
    Spec, Src0, Src1, C0, C1, Zero, select, lower, _has_src1,
)
from concourse.dve_uop import DveOpSpec
from concourse.tile import TileContext

N_CORES = 8
ROWS_TOTAL = 32 * 8192
E = 64  # experts per row
ROWS_PER_CORE = ROWS_TOTAL // N_CORES  # 32768
P = 128  # SBUF partitions
THRESHOLD = 0.2

TOT_FD = ROWS_PER_CORE * E // P  # 16384 f32 per partition
# graded tile schedule: small tiles at the ends for fast pipeline fill/drain
GRADED = (512, 512, 1024, 2048, 2048, 2048, 2048, 2048, 2048, 1024, 512, 512)
VARIANT = "fused"

_cached = None


def _register_fused_op():
    """Idempotently register the fused normalize+threshold DVE op:
    out = select(in0*in1 >= s0, in0*in1, 0)."""
    name = "SOFTMAX_THR_MASK_ANT"
    for op in dve_ops.OPS:
        if op.name == name:
            return op
    m = Src0 * Src1

    def _ref(in0, in1, s0, s1, imm2):
        mm = in0.astype(np.float32) * in1
        return np.where(mm >= s0, mm, 0.0).astype(np.float32)

    spec = Spec(body=select(m >= C0, m, Zero), reference=_ref)
    row = dve_ops._CUSTOM_DVE_ROW_BASE + len(dve_ops.OPS)
    shas = {}
    for ver in ("v3", "v4"):
        tmp = DveOpSpec(name=name, opcode=row, uops=lower(spec, ver=ver),
                        rd1_en=_has_src1(spec))
        shas[ver] = tmp.sha(ver)
    op = dve_ops.DveOp(name, spec, subdim=False, uops_sha=shas)
    dve_ops.OPS.append(op)
    dve_ops._SUB_OPCODE_FOR_NAME[name] = row
    dve_ops.CUSTOM_DVE_SPECS[name] = spec
    return op


def _register_fused_scale_op():
    """out = select(in0*in1 >= s0, in0*in1, 0) * s1 — for the pool_avg
    path where in1 = 64/s, s0 = 64*THRESHOLD, s1 = 1/64."""
    name = "SOFTMAX_THR_MASK_SCALE_ANT"
    for op in dve_ops.OPS:
        if op.name == name:
            return op
    m = Src0 * Src1

    def _ref(in0, in1, s0, s1, imm2):
        mm = in0.astype(np.float32) * in1
        return (np.where(mm >= s0, mm, 0.0) * s1).astype(np.float32)

    spec = Spec(body=select(m >= C0, m, Zero) * C1, reference=_ref)
    row = dve_ops._CUSTOM_DVE_ROW_BASE + len(dve_ops.OPS)
    shas = {}
    for ver in ("v3", "v4"):
        tmp = DveOpSpec(name=name, opcode=row, uops=lower(spec, ver=ver),
                        rd1_en=_has_src1(spec))
        shas[ver] = tmp.sha(ver)
    op = dve_ops.DveOp(name, spec, subdim=False, uops_sha=shas)
    dve_ops.OPS.append(op)
    dve_ops._SUB_OPCODE_FOR_NAME[name] = row
    dve_ops.CUSTOM_DVE_SPECS[name] = spec
    return op


FUSED_OP = _register_fused_op()
FUSED_SCALE_OP = _register_fused_scale_op()


def _build(hw_reps: int = 0, variant: str | None = None, bufs: int = 3,
           fds=GRADED):
    """Build the per-core program. hw_reps>0 wraps the body in a hardware
    For_i loop that re-runs it hw_reps times (for on-device timing only)."""
    variant = VARIANT if variant is None else variant
    assert sum(fds) == TOT_FD
    f32 = mybir.dt.float32
    nc = bacc.Bacc(
        "TRN2",
        target_bir_lowering=False,
        debug=False,
        num_devices=N_CORES,
    )
    x_d = nc.dram_tensor("x", [ROWS_PER_CORE * E], f32, kind="ExternalInput")
    o_d = nc.dram_tensor("o", [ROWS_PER_CORE * E], f32, kind="ExternalOutput")
    x_f = x_d.ap().rearrange("(p f) -> p f", p=P)
    o_f = o_d.ap().rearrange("(p f) -> p f", p=P)

    with TileContext(nc) as tc:
        with tc.tile_pool(name="work", bufs=bufs) as pool:

            def tile_fused(off, fd, do_dma=True, do_comp=True, use_pool=False):
                K = fd // E
                xt = pool.tile([P, fd], f32, tag="x", name="xt")
                if do_dma:
                    nc.sync.dma_start(xt[:], x_f[:, off:off + fd])
                if not do_comp:
                    nc.sync.dma_start(o_f[:, off:off + fd], xt[:])
                    return
                et = pool.tile([P, fd], f32, tag="e", name="et")
                nc.scalar.activation(
                    et[:], xt[:], mybir.ActivationFunctionType.Exp
                )
                e3 = et[:].rearrange("p (k c) -> p k c", c=E)
                st = pool.tile([P, K], f32, tag="s", name="st")
                if use_pool:
                    nc.vector.pool(st[:], e3, mybir.PoolFunctionType.avg)
                else:
                    nc.vector.reduce_sum(st[:], e3, axis=mybir.AxisListType.X)
                rt = pool.tile([P, K], f32, tag="r", name="rt")
                nc.vector.reciprocal(rt[:], st[:])
                ot = pool.tile([P, fd], f32, tag="o", name="ot")
                o3 = ot[:].rearrange("p (k c) -> p k c", c=E)
                if use_pool:
                    # rt = 64/s; mask at 64*thr then scale kept values by 1/64
                    nc.vector._custom_dve(
                        FUSED_SCALE_OP, out=o3, in0=e3,
                        in1=rt[:].broadcast_to([P, K, E]),
                        s0=E * THRESHOLD, s1=1.0 / E,
                    )
                else:
                    nc.vector._custom_dve(
                        FUSED_OP, out=o3, in0=e3,
                        in1=rt[:].broadcast_to([P, K, E]), s0=THRESHOLD,
                    )
                if do_dma:
                    nc.sync.dma_start(o_f[:, off:off + fd], ot[:])

            def tile_dve(off, fd):
                # previous-best 3-pass DVE variant (kept for comparison)
                K = fd // E
                xt = pool.tile([P, fd], f32, tag="x", name="xt")
                nc.sync.dma_start(xt[:], x_f[:, off:off + fd])
                et = pool.tile([P, fd], f32, tag="e", name="et")
                nc.scalar.activation(
                    et[:], xt[:], mybir.ActivationFunctionType.Exp
                )
                e3 = et[:].rearrange("p (k c) -> p k c", c=E)
                st = pool.tile([P, K], f32, tag="s", name="st")
                nc.vector.reduce_sum(st[:], e3, axis=mybir.AxisListType.X)
                rt = pool.tile([P, K], f32, tag="r", name="rt")
                nc.vector.reciprocal(rt[:], st[:])
                softt = pool.tile([P, fd], f32, tag="soft", name="softt")
                s3 = softt[:].rearrange("p (k c) -> p k c", c=E)
                ot = pool.tile([P, fd], f32, tag="o", name="ot")
                nc.vector.tensor_mul(
                    s3, e3, rt[:].broadcast_to([P, K, E])
                )
                nc.vector.scalar_tensor_tensor(
                    ot[:], softt[:], THRESHOLD, softt[:],
                    op0=mybir.AluOpType.is_ge, op1=mybir.AluOpType.mult,
                )
                nc.sync.dma_start(o_f[:, off:off + fd], ot[:])

            def body():
                off = 0
                for fd in fds:
                    if variant == "fused":
                        tile_fused(off, fd)
                    elif variant == "fusedpool":
                        tile_fused(off, fd, use_pool=True)
                    elif variant == "dve":
                        tile_dve(off, fd)
                    elif variant == "dmaonly":
                        tile_fused(off, fd, do_comp=False)
                    elif variant == "componly":
                        tile_fused(off, fd, do_dma=False)
                    elif variant == "componlypool":
                        tile_fused(off, fd, do_dma=False, use_pool=True)
                    else:
                        raise ValueError(variant)
                    off += fd

            if hw_reps > 0:
                with tc.For_i(0, hw_reps, 1):
                    body()
            else:
                body()
    nc.compile()
    return nc


def kernel(inputs: np.ndarray) -> np.ndarray:
    global _cached
    if _cached is None:
        _cached = _build()
    nc = _cached

    x = np.ascontiguousarray(inputs, dtype=np.float32).reshape(N_CORES, -1)
    in_maps = [{"x": x[c]} for c in range(N_CORES)]
    res = bass_utils.run_bass_kernel_spmd(nc, in_maps, core_ids=list(range(N_CORES)))
    out = np.concatenate([res.results[c]["o"] for c in range(N_CORES)])
    return out.reshape(inputs.shape).astype(np.float32, copy=False)


# revision 8
# speedup vs baseline: 1.1365x; 1.1365x over previous
"""Trainium2 Bass kernel for CustomSoftmaxExperts (topk_masking).

Math: reference computes softmax over the 64-expert axis, finds the 5th
largest softmax value per row, and keeps values >= max(kth, 0.2).
Since softmax rows sum to 1, at most 4 values can be >= 0.2, so any value
>= 0.2 is automatically within the top-5: the mask reduces EXACTLY to
``softmax >= 0.2`` (verified bit-identical against the jax reference).

Kernel per row (64 contiguous f32 in DRAM):
    e = exp(x)            # no max-subtract needed: |x| <= ~5.5, exp <= ~250
    s = sum(e); r = 1/s
    out  = (e*r >= 0.2) ? e*r : 0     # one fused custom-DVE pass

Sharding: 32*8192 = 262144 rows, data-parallel over 8 cores ->
32768 rows/core (8.39 MB in + 8.39 MB out per core; memory-bound,
per-core HBM roofline ~358 GB/s -> ~47 us).

Layout per core: flat [32768*64] viewed as [128 partitions x 16384],
tiled along free dim.  Engines: ACT exp; DVE segmented reduce_sum
[128,K,64]->[128,K], reciprocal, and ONE fused custom-DVE op
(soft = e*r; out = soft >= 0.2 ? soft : 0) via a runtime-registered
DveOp (SOFTMAX_THR_MASK_ANT) — halves DVE element passes vs separate
mul + scalar_tensor_tensor.
"""

import numpy as np

import concourse.bacc as bacc
import concourse.mybir as mybir
from concourse import bass_utils, dve_ops
from concourse.dve_spec import (
    Spec, Src0, Src1, C0, C1, Zero, select, lower, _has_src1,
)
from concourse.dve_uop import DveOpSpec
from concourse.tile import TileContext

N_CORES = 8
ROWS_TOTAL = 32 * 8192
E = 64  # experts per row
ROWS_PER_CORE = ROWS_TOTAL // N_CORES  # 32768
P = 128  # SBUF partitions
THRESHOLD = 0.2

TOT_FD = ROWS_PER_CORE * E // P  # 16384 f32 per partition
# graded tile schedule: small tiles at the ends for fast pipeline fill/drain
GRADED = (512, 512, 1024, 2048, 2048, 2048, 2048, 2048, 2048, 1024, 512, 512)
VARIANT = "fused"

_cached = None


def _register_fused_op():
    """Idempotently register the fused normalize+threshold DVE op:
    out = select(in0*in1 >= s0, in0*in1, 0)."""
    name = "SOFTMAX_THR_MASK_ANT"
    for op in dve_ops.OPS:
        if op.name == name:
            return op
    m = Src0 * Src1

    def _ref(in0, in1, s0, s1, imm2):
        mm = in0.astype(np.float32) * in1
        return np.where(mm >= s0, mm, 0.0).astype(np.float32)

    spec = Spec(body=select(m >= C0, m, Zero), reference=_ref)
    row = dve_ops._CUSTOM_DVE_ROW_BASE + len(dve_ops.OPS)
    shas = {}
    for ver in ("v3", "v4"):
        tmp = DveOpSpec(name=name, opcode=row, uops=lower(spec, ver=ver),
                        rd1_en=_has_src1(spec))
        shas[ver] = tmp.sha(ver)
    op = dve_ops.DveOp(name, spec, subdim=False, uops_sha=shas)
    dve_ops.OPS.append(op)
    dve_ops._SUB_OPCODE_FOR_NAME[name] = row
    dve_ops.CUSTOM_DVE_SPECS[name] = spec
    return op


def _register_fused_scale_op():
    """out = select(in0*in1 >= s0, in0*in1, 0) * s1 — for the pool_avg
    path where in1 = 64/s, s0 = 64*THRESHOLD, s1 = 1/64."""
    name = "SOFTMAX_THR_MASK_SCALE_ANT"
    for op in dve_ops.OPS:
        if op.name == name:
            return op
    m = Src0 * Src1

    def _ref(in0, in1, s0, s1, imm2):
        mm = in0.astype(np.float32) * in1
        return (np.where(mm >= s0, mm, 0.0) * s1).astype(np.float32)

    spec = Spec(body=select(m >= C0, m, Zero) * C1, reference=_ref)
    row = dve_ops._CUSTOM_DVE_ROW_BASE + len(dve_ops.OPS)
    shas = {}
    for ver in ("v3", "v4"):
        tmp = DveOpSpec(name=name, opcode=row, uops=lower(spec, ver=ver),
                        rd1_en=_has_src1(spec))
        shas[ver] = tmp.sha(ver)
    op = dve_ops.DveOp(name, spec, subdim=False, uops_sha=shas)
    dve_ops.OPS.append(op)
    dve_ops._SUB_OPCODE_FOR_NAME[name] = row
    dve_ops.CUSTOM_DVE_SPECS[name] = spec
    return op


FUSED_OP = _register_fused_op()
FUSED_SCALE_OP = _register_fused_scale_op()


def _build(hw_reps: int = 0, variant: str | None = None, bufs: int = 3,
           fds=GRADED, in_eng: str = "sync", out_eng: str = "sync"):
    """Build the per-core program. hw_reps>0 wraps the body in a hardware
    For_i loop that re-runs it hw_reps times (for on-device timing only).
    in_eng/out_eng pick the DMA-issuing engine: sync (SP ring), scalar
    (ACT ring), vector, or gpsimd (SWDGE)."""
    variant = VARIANT if variant is None else variant
    assert sum(fds) == TOT_FD
    f32 = mybir.dt.float32
    nc = bacc.Bacc(
        "TRN2",
        target_bir_lowering=False,
        debug=False,
        num_devices=N_CORES,
    )
    x_d = nc.dram_tensor("x", [ROWS_PER_CORE * E], f32, kind="ExternalInput")
    o_d = nc.dram_tensor("o", [ROWS_PER_CORE * E], f32, kind="ExternalOutput")
    x_f = x_d.ap().rearrange("(p f) -> p f", p=P)
    o_f = o_d.ap().rearrange("(p f) -> p f", p=P)

    in_dma = getattr(nc, in_eng).dma_start
    out_dma = getattr(nc, out_eng).dma_start

    with TileContext(nc) as tc:
        with tc.tile_pool(name="work", bufs=bufs) as pool:

            def tile_fused(off, fd, do_dma=True, do_comp=True, use_pool=False,
                           do_out=True):
                K = fd // E
                xt = pool.tile([P, fd], f32, tag="x", name="xt")
                if do_dma:
                    in_dma(xt[:], x_f[:, off:off + fd])
                if not do_comp:
                    out_dma(o_f[:, off:off + fd], xt[:])
                    return
                et = pool.tile([P, fd], f32, tag="e", name="et")
                nc.scalar.activation(
                    et[:], xt[:], mybir.ActivationFunctionType.Exp
                )
                e3 = et[:].rearrange("p (k c) -> p k c", c=E)
                st = pool.tile([P, K], f32, tag="s", name="st")
                if use_pool:
                    nc.vector.pool(st[:], e3, mybir.PoolFunctionType.avg)
                else:
                    nc.vector.reduce_sum(st[:], e3, axis=mybir.AxisListType.X)
                rt = pool.tile([P, K], f32, tag="r", name="rt")
                nc.vector.reciprocal(rt[:], st[:])
                ot = pool.tile([P, fd], f32, tag="o", name="ot")
                o3 = ot[:].rearrange("p (k c) -> p k c", c=E)
                if use_pool:
                    # rt = 64/s; mask at 64*thr then scale kept values by 1/64
                    nc.vector._custom_dve(
                        FUSED_SCALE_OP, out=o3, in0=e3,
                        in1=rt[:].broadcast_to([P, K, E]),
                        s0=E * THRESHOLD, s1=1.0 / E,
                    )
                else:
                    nc.vector._custom_dve(
                        FUSED_OP, out=o3, in0=e3,
                        in1=rt[:].broadcast_to([P, K, E]), s0=THRESHOLD,
                    )
                if do_dma and do_out:
                    out_dma(o_f[:, off:off + fd], ot[:])

            def tile_dve(off, fd):
                # previous-best 3-pass DVE variant (kept for comparison)
                K = fd // E
                xt = pool.tile([P, fd], f32, tag="x", name="xt")
                nc.sync.dma_start(xt[:], x_f[:, off:off + fd])
                et = pool.tile([P, fd], f32, tag="e", name="et")
                nc.scalar.activation(
                    et[:], xt[:], mybir.ActivationFunctionType.Exp
                )
                e3 = et[:].rearrange("p (k c) -> p k c", c=E)
                st = pool.tile([P, K], f32, tag="s", name="st")
                nc.vector.reduce_sum(st[:], e3, axis=mybir.AxisListType.X)
                rt = pool.tile([P, K], f32, tag="r", name="rt")
                nc.vector.reciprocal(rt[:], st[:])
                softt = pool.tile([P, fd], f32, tag="soft", name="softt")
                s3 = softt[:].rearrange("p (k c) -> p k c", c=E)
                ot = pool.tile([P, fd], f32, tag="o", name="ot")
                nc.vector.tensor_mul(
                    s3, e3, rt[:].broadcast_to([P, K, E])
                )
                nc.vector.scalar_tensor_tensor(
                    ot[:], softt[:], THRESHOLD, softt[:],
                    op0=mybir.AluOpType.is_ge, op1=mybir.AluOpType.mult,
                )
                nc.sync.dma_start(o_f[:, off:off + fd], ot[:])

            def body():
                off = 0
                for fd in fds:
                    if variant == "fused":
                        tile_fused(off, fd)
                    elif variant == "fusedpool":
                        tile_fused(off, fd, use_pool=True)
                    elif variant == "dve":
                        tile_dve(off, fd)
                    elif variant == "dmaonly":
                        tile_fused(off, fd, do_comp=False)
                    elif variant == "compin":
                        tile_fused(off, fd, do_out=False)
                    elif variant == "compinpool":
                        tile_fused(off, fd, do_out=False, use_pool=True)
                    else:
                        raise ValueError(variant)
                    off += fd

            if hw_reps > 0:
                with tc.For_i(0, hw_reps, 1):
                    body()
            else:
                body()
    nc.compile()
    return nc


def kernel(inputs: np.ndarray) -> np.ndarray:
    global _cached
    if _cached is None:
        _cached = _build()
    nc = _cached

    x = np.ascontiguousarray(inputs, dtype=np.float32).reshape(N_CORES, -1)
    in_maps = [{"x": x[c]} for c in range(N_CORES)]
    res = bass_utils.run_bass_kernel_spmd(nc, in_maps, core_ids=list(range(N_CORES)))
    out = np.concatenate([res.results[c]["o"] for c in range(N_CORES)])
    return out.reshape(inputs.shape).astype(np.float32, copy=False)


# revision 10
# speedup vs baseline: 1.1588x; 1.0197x over previous
"""Trainium2 Bass kernel for CustomSoftmaxExperts (topk_masking).

Math: reference computes softmax over the 64-expert axis, finds the 5th
largest softmax value per row, and keeps values >= max(kth, 0.2).
Since softmax rows sum to 1, at most 4 values can be >= 0.2, so any value
>= 0.2 is automatically within the top-5: the mask reduces EXACTLY to
``softmax >= 0.2`` (verified bit-identical against the jax reference).

Kernel per row (64 contiguous f32 in DRAM):
    e = exp(x)            # no max-subtract needed: |x| <= ~5.5, exp <= ~250
    s = sum(e); r = 1/s
    out  = (e*r >= 0.2) ? e*r : 0     # one fused custom-DVE pass

Sharding: 32*8192 = 262144 rows, data-parallel over 8 cores ->
32768 rows/core (8.39 MB in + 8.39 MB out per core; memory-bound,
per-core HBM roofline ~358 GB/s -> ~47 us).

Layout per core: flat [32768*64] viewed as [128 partitions x 16384],
tiled along free dim.  Engines: ACT exp; DVE segmented reduce_sum
[128,K,64]->[128,K], reciprocal, and ONE fused custom-DVE op
(soft = e*r; out = soft >= 0.2 ? soft : 0) via a runtime-registered
DveOp (SOFTMAX_THR_MASK_ANT) — halves DVE element passes vs separate
mul + scalar_tensor_tensor.
"""

import numpy as np

import concourse.bacc as bacc
import concourse.mybir as mybir
from concourse import bass_utils, dve_ops
from concourse.dve_spec import (
    Spec, Src0, Src1, C0, C1, Zero, select, lower, _has_src1,
)
from concourse.dve_uop import DveOpSpec
from concourse.tile import TileContext

N_CORES = 8
ROWS_TOTAL = 32 * 8192
E = 64  # experts per row
ROWS_PER_CORE = ROWS_TOTAL // N_CORES  # 32768
P = 128  # SBUF partitions
THRESHOLD = 0.2

TOT_FD = ROWS_PER_CORE * E // P  # 16384 f32 per partition
# graded tile schedule: small tiles at the ends for fast pipeline fill/drain
GRADED = (512, 512, 1024, 2048, 2048, 2048, 2048, 2048, 2048, 1024, 512, 512)
VARIANT = "fused"

_cached = None


def _register_fused_op():
    """Idempotently register the fused normalize+threshold DVE op:
    out = select(in0*in1 >= s0, in0*in1, 0)."""
    name = "SOFTMAX_THR_MASK_ANT"
    for op in dve_ops.OPS:
        if op.name == name:
            return op
    m = Src0 * Src1

    def _ref(in0, in1, s0, s1, imm2):
        mm = in0.astype(np.float32) * in1
        return np.where(mm >= s0, mm, 0.0).astype(np.float32)

    spec = Spec(body=select(m >= C0, m, Zero), reference=_ref)
    row = dve_ops._CUSTOM_DVE_ROW_BASE + len(dve_ops.OPS)
    shas = {}
    for ver in ("v3", "v4"):
        tmp = DveOpSpec(name=name, opcode=row, uops=lower(spec, ver=ver),
                        rd1_en=_has_src1(spec))
        shas[ver] = tmp.sha(ver)
    op = dve_ops.DveOp(name, spec, subdim=False, uops_sha=shas)
    dve_ops.OPS.append(op)
    dve_ops._SUB_OPCODE_FOR_NAME[name] = row
    dve_ops.CUSTOM_DVE_SPECS[name] = spec
    return op


def _register_fused_scale_op():
    """out = select(in0*in1 >= s0, in0*in1, 0) * s1 — for the pool_avg
    path where in1 = 64/s, s0 = 64*THRESHOLD, s1 = 1/64."""
    name = "SOFTMAX_THR_MASK_SCALE_ANT"
    for op in dve_ops.OPS:
        if op.name == name:
            return op
    m = Src0 * Src1

    def _ref(in0, in1, s0, s1, imm2):
        mm = in0.astype(np.float32) * in1
        return (np.where(mm >= s0, mm, 0.0) * s1).astype(np.float32)

    spec = Spec(body=select(m >= C0, m, Zero) * C1, reference=_ref)
    row = dve_ops._CUSTOM_DVE_ROW_BASE + len(dve_ops.OPS)
    shas = {}
    for ver in ("v3", "v4"):
        tmp = DveOpSpec(name=name, opcode=row, uops=lower(spec, ver=ver),
                        rd1_en=_has_src1(spec))
        shas[ver] = tmp.sha(ver)
    op = dve_ops.DveOp(name, spec, subdim=False, uops_sha=shas)
    dve_ops.OPS.append(op)
    dve_ops._SUB_OPCODE_FOR_NAME[name] = row
    dve_ops.CUSTOM_DVE_SPECS[name] = spec
    return op


FUSED_OP = _register_fused_op()
FUSED_SCALE_OP = _register_fused_scale_op()


def _build(hw_reps: int = 0, variant: str | None = None, bufs: int = 3,
           fds=GRADED, in_eng: str = "sync", out_eng: str = "sync",
           recipfast: bool = True, out_split: int = 1,
           in_pat: str | None = None, out_pat: str | None = None):
    """Build the per-core program. hw_reps>0 wraps the body in a hardware
    For_i loop that re-runs it hw_reps times (for on-device timing only).
    in_eng/out_eng pick the DMA-issuing engine: sync (SP ring), scalar
    (ACT ring), vector, or gpsimd (SWDGE)."""
    variant = VARIANT if variant is None else variant
    assert sum(fds) == TOT_FD
    f32 = mybir.dt.float32
    nc = bacc.Bacc(
        "TRN2",
        target_bir_lowering=False,
        debug=False,
        num_devices=N_CORES,
    )
    x_d = nc.dram_tensor("x", [ROWS_PER_CORE * E], f32, kind="ExternalInput")
    o_d = nc.dram_tensor("o", [ROWS_PER_CORE * E], f32, kind="ExternalOutput")
    x_f = x_d.ap().rearrange("(p f) -> p f", p=P)
    o_f = o_d.ap().rearrange("(p f) -> p f", p=P)

    ENG = {"s": "sync", "a": "scalar", "g": "gpsimd"}

    def _pat_dma(pat, fallback):
        def f(idx):
            eng = ENG[pat[idx % len(pat)]] if pat else fallback
            return getattr(nc, eng).dma_start
        return f

    in_dma_for = _pat_dma(in_pat, in_eng)
    out_dma_for = _pat_dma(out_pat, out_eng)

    with TileContext(nc) as tc:
        with tc.tile_pool(name="work", bufs=bufs) as pool:

            def tile_fused(idx, off, fd, do_dma=True, do_comp=True,
                           use_pool=False, do_out=True):
                K = fd // E
                in_dma = in_dma_for(idx)
                out_dma = out_dma_for(idx)
                xt = pool.tile([P, fd], f32, tag="x", name="xt")
                if do_dma:
                    in_dma(xt[:], x_f[:, off:off + fd])
                if not do_comp:
                    out_dma(o_f[:, off:off + fd], xt[:])
                    return
                et = pool.tile([P, fd], f32, tag="e", name="et")
                nc.scalar.activation(
                    et[:], xt[:], mybir.ActivationFunctionType.Exp
                )
                e3 = et[:].rearrange("p (k c) -> p k c", c=E)
                st = pool.tile([P, K], f32, tag="s", name="st")
                if use_pool:
                    nc.vector.pool(st[:], e3, mybir.PoolFunctionType.avg)
                else:
                    nc.vector.reduce_sum(st[:], e3, axis=mybir.AxisListType.X)
                rt = pool.tile([P, K], f32, tag="r", name="rt")
                if recipfast:
                    nc.vector.reciprocal_approx_fast(out=rt[:], in_=st[:])
                else:
                    nc.vector.reciprocal(rt[:], st[:])
                ot = pool.tile([P, fd], f32, tag="o", name="ot")
                o3 = ot[:].rearrange("p (k c) -> p k c", c=E)
                if use_pool:
                    # rt = 64/s; mask at 64*thr then scale kept values by 1/64
                    nc.vector._custom_dve(
                        FUSED_SCALE_OP, out=o3, in0=e3,
                        in1=rt[:].broadcast_to([P, K, E]),
                        s0=E * THRESHOLD, s1=1.0 / E,
                    )
                else:
                    nc.vector._custom_dve(
                        FUSED_OP, out=o3, in0=e3,
                        in1=rt[:].broadcast_to([P, K, E]), s0=THRESHOLD,
                    )
                if do_dma and do_out:
                    step = fd // out_split
                    for j in range(out_split):
                        out_dma(o_f[:, off + j * step:off + (j + 1) * step],
                                ot[:, j * step:(j + 1) * step])

            def tile_dve(off, fd):
                # previous-best 3-pass DVE variant (kept for comparison)
                K = fd // E
                xt = pool.tile([P, fd], f32, tag="x", name="xt")
                nc.sync.dma_start(xt[:], x_f[:, off:off + fd])
                et = pool.tile([P, fd], f32, tag="e", name="et")
                nc.scalar.activation(
                    et[:], xt[:], mybir.ActivationFunctionType.Exp
                )
                e3 = et[:].rearrange("p (k c) -> p k c", c=E)
                st = pool.tile([P, K], f32, tag="s", name="st")
                nc.vector.reduce_sum(st[:], e3, axis=mybir.AxisListType.X)
                rt = pool.tile([P, K], f32, tag="r", name="rt")
                nc.vector.reciprocal(rt[:], st[:])
                softt = pool.tile([P, fd], f32, tag="soft", name="softt")
                s3 = softt[:].rearrange("p (k c) -> p k c", c=E)
                ot = pool.tile([P, fd], f32, tag="o", name="ot")
                nc.vector.tensor_mul(
                    s3, e3, rt[:].broadcast_to([P, K, E])
                )
                nc.vector.scalar_tensor_tensor(
                    ot[:], softt[:], THRESHOLD, softt[:],
                    op0=mybir.AluOpType.is_ge, op1=mybir.AluOpType.mult,
                )
                nc.sync.dma_start(o_f[:, off:off + fd], ot[:])

            def body():
                off = 0
                for idx, fd in enumerate(fds):
                    if variant == "fused":
                        tile_fused(idx, off, fd)
                    elif variant == "fusedpool":
                        tile_fused(idx, off, fd, use_pool=True)
                    elif variant == "dve":
                        tile_dve(off, fd)
                    elif variant == "dmaonly":
                        tile_fused(idx, off, fd, do_comp=False)
                    elif variant == "compin":
                        tile_fused(idx, off, fd, do_out=False)
                    elif variant == "compinpool":
                        tile_fused(idx, off, fd, do_out=False, use_pool=True)
                    else:
                        raise ValueError(variant)
                    off += fd

            if hw_reps > 0:
                with tc.For_i(0, hw_reps, 1):
                    body()
            else:
                body()
    nc.compile()
    return nc


def kernel(inputs: np.ndarray) -> np.ndarray:
    global _cached
    if _cached is None:
        _cached = _build()
    nc = _cached

    x = np.ascontiguousarray(inputs, dtype=np.float32).reshape(N_CORES, -1)
    in_maps = [{"x": x[c]} for c in range(N_CORES)]
    res = bass_utils.run_bass_kernel_spmd(nc, in_maps, core_ids=list(range(N_CORES)))
    out = np.concatenate([res.results[c]["o"] for c in range(N_CORES)])
    return out.reshape(inputs.shape).astype(np.float32, copy=False)


# revision 12
# speedup vs baseline: 1.2958x; 1.1182x over previous
"""Trainium2 Bass kernel for CustomSoftmaxExperts (topk_masking).

Math: reference computes softmax over the 64-expert axis, finds the 5th
largest softmax value per row, and keeps values >= max(kth, 0.2).
Since softmax rows sum to 1, at most 4 values can be >= 0.2, so any value
>= 0.2 is automatically within the top-5: the mask reduces EXACTLY to
``softmax >= 0.2`` (verified bit-identical against the jax reference).

Kernel per row (64 contiguous f32 in DRAM):
    e = exp(x)            # no max-subtract needed: |x| <= ~5.5, exp <= ~250
    s = sum(e); r = 1/s
    out  = (e*r >= 0.2) ? e*r : 0     # one fused custom-DVE pass

Sharding: 32*8192 = 262144 rows, data-parallel over 8 cores ->
32768 rows/core (8.39 MB in + 8.39 MB out per core; memory-bound,
per-core HBM roofline ~358 GB/s -> ~47 us).

Layout per core: flat [32768*64] viewed as [128 partitions x 16384],
tiled along free dim.  Engines: ACT exp; DVE segmented reduce_sum
[128,K,64]->[128,K], reciprocal, and ONE fused custom-DVE op
(soft = e*r; out = soft >= 0.2 ? soft : 0) via a runtime-registered
DveOp (SOFTMAX_THR_MASK_ANT) — halves DVE element passes vs separate
mul + scalar_tensor_tensor.
"""

import numpy as np

import concourse.bacc as bacc
import concourse.mybir as mybir
from concourse import bass_utils, dve_ops
from concourse.dve_spec import (
    Spec, Src0, Src1, C0, C1, Zero, select, lower, _has_src1,
)
from concourse.dve_uop import DveOpSpec
from concourse.tile import TileContext

N_CORES = 8
ROWS_TOTAL = 32 * 8192
E = 64  # experts per row
ROWS_PER_CORE = ROWS_TOTAL // N_CORES  # 32768
P = 128  # SBUF partitions
THRESHOLD = 0.2

TOT_FD = ROWS_PER_CORE * E // P  # 16384 f32 per partition
# graded tile schedule: small tiles at the ends for fast pipeline fill/drain
GRADED = (512, 512, 1024, 2048, 2048, 2048, 2048, 2048, 2048, 1024, 512, 512)
VARIANT = "fused"

_cached = None


def _register_fused_op():
    """Idempotently register the fused normalize+threshold DVE op:
    out = select(in0*in1 >= s0, in0*in1, 0)."""
    name = "SOFTMAX_THR_MASK_ANT"
    for op in dve_ops.OPS:
        if op.name == name:
            return op
    m = Src0 * Src1

    def _ref(in0, in1, s0, s1, imm2):
        mm = in0.astype(np.float32) * in1
        return np.where(mm >= s0, mm, 0.0).astype(np.float32)

    spec = Spec(body=select(m >= C0, m, Zero), reference=_ref)
    row = dve_ops._CUSTOM_DVE_ROW_BASE + len(dve_ops.OPS)
    shas = {}
    for ver in ("v3", "v4"):
        tmp = DveOpSpec(name=name, opcode=row, uops=lower(spec, ver=ver),
                        rd1_en=_has_src1(spec))
        shas[ver] = tmp.sha(ver)
    op = dve_ops.DveOp(name, spec, subdim=False, uops_sha=shas)
    dve_ops.OPS.append(op)
    dve_ops._SUB_OPCODE_FOR_NAME[name] = row
    dve_ops.CUSTOM_DVE_SPECS[name] = spec
    return op


def _register_fused_scale_op():
    """out = select(in0*in1 >= s0, in0*in1, 0) * s1 — for the pool_avg
    path where in1 = 64/s, s0 = 64*THRESHOLD, s1 = 1/64."""
    name = "SOFTMAX_THR_MASK_SCALE_ANT"
    for op in dve_ops.OPS:
        if op.name == name:
            return op
    m = Src0 * Src1

    def _ref(in0, in1, s0, s1, imm2):
        mm = in0.astype(np.float32) * in1
        return (np.where(mm >= s0, mm, 0.0) * s1).astype(np.float32)

    spec = Spec(body=select(m >= C0, m, Zero) * C1, reference=_ref)
    row = dve_ops._CUSTOM_DVE_ROW_BASE + len(dve_ops.OPS)
    shas = {}
    for ver in ("v3", "v4"):
        tmp = DveOpSpec(name=name, opcode=row, uops=lower(spec, ver=ver),
                        rd1_en=_has_src1(spec))
        shas[ver] = tmp.sha(ver)
    op = dve_ops.DveOp(name, spec, subdim=False, uops_sha=shas)
    dve_ops.OPS.append(op)
    dve_ops._SUB_OPCODE_FOR_NAME[name] = row
    dve_ops.CUSTOM_DVE_SPECS[name] = spec
    return op


FUSED_OP = _register_fused_op()
FUSED_SCALE_OP = _register_fused_scale_op()


def _build(hw_reps: int = 0, variant: str | None = None, bufs: int = 3,
           fds=GRADED, in_eng: str = "sync", out_eng: str = "sync",
           recipfast: bool = True, out_split: int = 1,
           in_pat: str | None = None, out_pat: str | None = None,
           tag_bufs: dict | None = None, gp_frac: float = 0.0):
    """Build the per-core program. hw_reps>0 wraps the body in a hardware
    For_i loop that re-runs it hw_reps times (for on-device timing only).
    in_eng/out_eng pick the DMA-issuing engine: sync (SP ring), scalar
    (ACT ring), vector, or gpsimd (SWDGE)."""
    variant = VARIANT if variant is None else variant
    assert sum(fds) == TOT_FD
    f32 = mybir.dt.float32
    nc = bacc.Bacc(
        "TRN2",
        target_bir_lowering=False,
        debug=False,
        num_devices=N_CORES,
    )
    x_d = nc.dram_tensor("x", [ROWS_PER_CORE * E], f32, kind="ExternalInput")
    o_d = nc.dram_tensor("o", [ROWS_PER_CORE * E], f32, kind="ExternalOutput")
    x_f = x_d.ap().rearrange("(p f) -> p f", p=P)
    o_f = o_d.ap().rearrange("(p f) -> p f", p=P)

    ENG = {"s": "sync", "a": "scalar", "g": "gpsimd"}

    def _pat_dma(pat, fallback):
        def f(idx):
            eng = ENG[pat[idx % len(pat)]] if pat else fallback
            return getattr(nc, eng).dma_start
        return f

    in_dma_for = _pat_dma(in_pat, in_eng)
    out_dma_for = _pat_dma(out_pat, out_eng)

    with TileContext(nc) as tc:
        with tc.tile_pool(name="work", bufs=bufs) as pool:

            def tile_fused(idx, off, fd, do_dma=True, do_comp=True,
                           use_pool=False, do_out=True):
                K = fd // E
                in_dma = in_dma_for(idx)
                out_dma = out_dma_for(idx)
                tb = tag_bufs or {}
                xt = pool.tile([P, fd], f32, tag="x", name="xt",
                               bufs=tb.get("x"))
                if do_dma:
                    in_dma(xt[:], x_f[:, off:off + fd])
                if not do_comp:
                    out_dma(o_f[:, off:off + fd], xt[:])
                    return
                et = pool.tile([P, fd], f32, tag="e", name="et",
                               bufs=tb.get("e"))
                nc.scalar.activation(
                    et[:], xt[:], mybir.ActivationFunctionType.Exp
                )
                e3 = et[:].rearrange("p (k c) -> p k c", c=E)
                st = pool.tile([P, K], f32, tag="s", name="st")
                if use_pool:
                    nc.vector.pool(st[:], e3, mybir.PoolFunctionType.avg)
                else:
                    nc.vector.reduce_sum(st[:], e3, axis=mybir.AxisListType.X)
                rt = pool.tile([P, K], f32, tag="r", name="rt")
                if recipfast:
                    nc.vector.reciprocal_approx_fast(out=rt[:], in_=st[:])
                else:
                    nc.vector.reciprocal(rt[:], st[:])
                ot = pool.tile([P, fd], f32, tag="o", name="ot",
                               bufs=tb.get("o"))
                o3 = ot[:].rearrange("p (k c) -> p k c", c=E)
                kd = int(K * gp_frac + 0.5) if gp_frac > 0 else 0
                if kd > 0:
                    # rows [0,kd) on GPSIMD: 3-pass mul + mask via soft tile
                    sgt = pool.tile([P, kd * E], f32, tag="sg", name="sgt",
                                    bufs=tb.get("sg"))
                    sg3 = sgt[:].rearrange("p (k c) -> p k c", c=E)
                    nc.gpsimd.tensor_mul(
                        sg3, e3[:, 0:kd],
                        rt[:, 0:kd].broadcast_to([P, kd, E]),
                    )
                    nc.gpsimd.scalar_tensor_tensor(
                        o3[:, 0:kd], sg3, THRESHOLD, sg3,
                        op0=mybir.AluOpType.is_ge, op1=mybir.AluOpType.mult,
                    )
                if use_pool:
                    # rt = 64/s; mask at 64*thr then scale kept values by 1/64
                    nc.vector._custom_dve(
                        FUSED_SCALE_OP, out=o3[:, kd:K], in0=e3[:, kd:K],
                        in1=rt[:, kd:K].broadcast_to([P, K - kd, E]),
                        s0=E * THRESHOLD, s1=1.0 / E,
                    )
                else:
                    nc.vector._custom_dve(
                        FUSED_OP, out=o3[:, kd:K], in0=e3[:, kd:K],
                        in1=rt[:, kd:K].broadcast_to([P, K - kd, E]),
                        s0=THRESHOLD,
                    )
                if do_dma and do_out:
                    step = fd // out_split
                    for j in range(out_split):
                        out_dma(o_f[:, off + j * step:off + (j + 1) * step],
                                ot[:, j * step:(j + 1) * step])

            def tile_dve(off, fd):
                # previous-best 3-pass DVE variant (kept for comparison)
                K = fd // E
                xt = pool.tile([P, fd], f32, tag="x", name="xt")
                nc.sync.dma_start(xt[:], x_f[:, off:off + fd])
                et = pool.tile([P, fd], f32, tag="e", name="et")
                nc.scalar.activation(
                    et[:], xt[:], mybir.ActivationFunctionType.Exp
                )
                e3 = et[:].rearrange("p (k c) -> p k c", c=E)
                st = pool.tile([P, K], f32, tag="s", name="st")
                nc.vector.reduce_sum(st[:], e3, axis=mybir.AxisListType.X)
                rt = pool.tile([P, K], f32, tag="r", name="rt")
                nc.vector.reciprocal(rt[:], st[:])
                softt = pool.tile([P, fd], f32, tag="soft", name="softt")
                s3 = softt[:].rearrange("p (k c) -> p k c", c=E)
                ot = pool.tile([P, fd], f32, tag="o", name="ot")
                nc.vector.tensor_mul(
                    s3, e3, rt[:].broadcast_to([P, K, E])
                )
                nc.vector.scalar_tensor_tensor(
                    ot[:], softt[:], THRESHOLD, softt[:],
                    op0=mybir.AluOpType.is_ge, op1=mybir.AluOpType.mult,
                )
                nc.sync.dma_start(o_f[:, off:off + fd], ot[:])

            def body():
                off = 0
                for idx, fd in enumerate(fds):
                    if variant == "fused":
                        tile_fused(idx, off, fd)
                    elif variant == "fusedpool":
                        tile_fused(idx, off, fd, use_pool=True)
                    elif variant == "dve":
                        tile_dve(off, fd)
                    elif variant == "dmaonly":
                        tile_fused(idx, off, fd, do_comp=False)
                    elif variant == "compin":
                        tile_fused(idx, off, fd, do_out=False)
                    elif variant == "compinpool":
                        tile_fused(idx, off, fd, do_out=False, use_pool=True)
                    else:
                        raise ValueError(variant)
                    off += fd

            if hw_reps > 0:
                with tc.For_i(0, hw_reps, 1):
                    body()
            else:
                body()
    nc.compile()
    return nc


def kernel(inputs: np.ndarray) -> np.ndarray:
    global _cached
    if _cached is None:
        _cached = _build()
    nc = _cached

    x = np.ascontiguousarray(inputs, dtype=np.float32).reshape(N_CORES, -1)
    in_maps = [{"x": x[c]} for c in range(N_CORES)]
    res = bass_utils.run_bass_kernel_spmd(nc, in_maps, core_ids=list(range(N_CORES)))
    out = np.concatenate([res.results[c]["o"] for c in range(N_CORES)])
    return out.reshape(inputs.shape).astype(np.float32, copy=False)
